# revision 38
# baseline (speedup 1.0000x reference)
"""Trainium2 Bass kernel for nn_CMFuser (topk_masking) — v5.

Self-contained: accepts FULL inputs (as produced by setup_inputs()), returns
the FULL [32, 512, 768] output. Internally shards batch across 8 NeuronCores
(pure data parallel, 4 batches/core) and runs a hand-written Bass/Tile kernel.

Algorithmic structure (validated against the jax reference):
  * BN(eval) + topk-channel-exchange blend folds into per-channel affine:
        x0_rgb = A1*rgb + A2*depth + A3,   x0_depth = D1*depth + D2*rgb + D3
  * The 2-token attention with -1e9 diag mask is an EXACT token swap, so
    qkv+softmax+proj collapse into one fused C x C matmul Wc = proj_w @ Wv
    applied to the OTHER token.
  * LN weights fold into the following matmul; LN mean-subtraction folds
    into a rank-1 (K=1) matmul correction (norm1) / bcast subtract (norm2).
  * Final LN + mean over the 2 modality tokens folds into 0.5*wf scale.

v5 performance structure (vs the 417us v2.2 baseline):
  * Host ships inputs CHANNEL-MAJOR fp16 and takes the output back
    channel-major fp16 (layout is our choice per the sharding contract), so
    the kernel has ZERO PE transposes and no output copy chain; the final
    tiles DMA straight from SBUF to HBM.
  * Wc runs a SINGLE fp8 DoubleRow pass (the hi/lo compensation pass was
    dropped; measured rel-err 1.66e-2 < 2e-2 gate).
  * LN row chain shortened: std16 = sqrt(u/C + eps) in ONE activation, an
    fp16 DVE reciprocal, packed per-stat rows; broadcasts are K=1 PE
    matmuls + DVE copies from a packed row (a DMA DRAM-bounce broadcast
    with a 0-stride AP also works — see bcast_pack history — but measured
    ~7us slower end-to-end due to chain latency).
  * Elementwise work spread Pool/DVE/ACT by measured tile-sim cost (Pool
    flat 427ns any dtype, no PSUM access allowed; DVE 327ns fp16-SBUF,
    658ns fp32/PSUM; ACT 612ns + 1283ns activation-table swaps, so Sqrt
    ops are clustered away from the GELU blocks).
  * Deep software pipeline: S1 stats run TWO groups ahead, W is split into
    apply + per-stream matmul stages, and Mtail is emitted in single-co
    chunks used as PE fillers under the LN row chains (PE executes largely
    in order, so emission order controls what covers each stall).
Measured: 380761 ns tile-sim span, rel err 1.658e-2 (gate: 2e-2).
"""

import os
import sys

sys.path.insert(0, "/opt/trn_rl_repo")

import numpy as np
import ml_dtypes

import concourse.bass as bass
import concourse.mybir as mybir
import concourse.tile as tile
from contextlib import ExitStack

dt = mybir.dt
Alu = mybir.AluOpType
Act = mybir.ActivationFunctionType
PerfMode = mybir.MatmulPerfMode

B, T, C = 32, 512, 768
H = 4
K_EX = int(C * 0.2)
MLP = 4 * C
EPS = 1e-5
N_CORES = 8
B_CORE = B // N_CORES          # 4 batches per core
ROWS = B_CORE * T              # 2048 token-sites per core
TG = 512                       # tokens per group
NG = ROWS // TG                # 4 groups per core
CT = C // 128                  # 6 channel tiles
CP = CT // 2                   # 3 channel k-pairs (DoubleRow)
MT = MLP // 128                # 24 mlp tiles
MP = MT // 2                   # 12 mlp k-pairs
WSCALE = 16.0                  # fp8 weight pre-scale (descaled on device)
WC_SINGLE = True              # drop the Wc lo-pass (accuracy experiment)

# vector slot indices in the packed per-channel constant table
V_A1, V_A2, V_A3, V_D1, V_D2, V_D3, V_SCLW, V_SCL, V_WFH, V_BF = range(10)
NV = 10

_CACHE = {}


def _build_nc(legalize=True):
    """Build the per-core Bass module (same program on all 8 cores)."""
    nc = bass.Bass()

    rgb_d = nc.dram_tensor("rgb", [C, ROWS], dt.float16, kind="ExternalInput")
    dep_d = nc.dram_tensor("dep", [C, ROWS], dt.float16, kind="ExternalInput")
    # per-stat-call scratch rows for the DMA partition-broadcast bounce
    scr_d = nc.dram_tensor("scr", [3 * NG, 4 * TG], dt.float16,
                           kind="Internal")
    wc_d = nc.dram_tensor("wc", [128, CP * 2 * C], dt.float8e4,
                          kind="ExternalInput")
    fc1_d = nc.dram_tensor("fc1", [128, CP * 2 * MLP], dt.float8e4,
                           kind="ExternalInput")
    fc2_d = nc.dram_tensor("fc2", [128, MP * 2 * C], dt.float8e4,
                           kind="ExternalInput")
    vecs_d = nc.dram_tensor("vecs", [128, CT * NV], dt.float32,
                            kind="ExternalInput")
    fb1_d = nc.dram_tensor("fb1", [128, MT], dt.float32, kind="ExternalInput")
    wcsum_d = nc.dram_tensor("wcsum", [1, C], dt.bfloat16, kind="ExternalInput")
    out_d = nc.dram_tensor("out", [C, ROWS], dt.float16, kind="ExternalOutput")

    with tile.TileContext(nc) as tc, ExitStack() as ctx:
        const = ctx.enter_context(tc.tile_pool(name="const", bufs=1))
        inp = ctx.enter_context(tc.tile_pool(name="inp", bufs=22))
        xp = ctx.enter_context(tc.tile_pool(name="xp", bufs=28))
        sqp = ctx.enter_context(tc.tile_pool(name="sqp", bufs=12))
        h1p = ctx.enter_context(tc.tile_pool(name="h1p", bufs=13))
        h2p = ctx.enter_context(tc.tile_pool(name="h2p", bufs=7))
        a8p = ctx.enter_context(tc.tile_pool(name="a8p", bufs=24))
        bcp = ctx.enter_context(tc.tile_pool(name="bcp", bufs=2))
        tmpp = ctx.enter_context(tc.tile_pool(name="tmpp", bufs=9))
        rows = ctx.enter_context(tc.tile_pool(name="rows", bufs=4))
        uap = ctx.enter_context(tc.tile_pool(name="uap", bufs=6))
        psum = ctx.enter_context(tc.tile_pool(name="psum", bufs=2, space="PSUM"))

        # ---- constants / weights; all input DMAs prefetched upfront ----
        vecs_sb = const.tile([128, CT * NV], dt.float32)
        nc.sync.dma_start(vecs_sb[:], vecs_d[:])
        fb1_sb = const.tile([128, MT], dt.float32)
        nc.sync.dma_start(fb1_sb[:], fb1_d[:])
        wcsum_sb = const.tile([1, C], dt.bfloat16)
        nc.sync.dma_start(wcsum_sb[:], wcsum_d[:])

        in_tiles = [dict() for _ in range(NG)]

        def dma_group_inputs(g):
            r0 = g * TG
            for s_, src_ in ((0, rgb_d), (1, dep_d)):
                for j_ in range(CT):
                    it_ = inp.tile([128, TG], dt.float16, tag="in",
                                   name=f"in_{g}_{s_}_{j_}")
                    nc.sync.dma_start(
                        it_[:], src_[j_ * 128:(j_ + 1) * 128, r0:r0 + TG])
                    in_tiles[g][s_, j_] = it_

        dma_group_inputs(0)
        wc_sb = const.tile([128, CP, 2, C], dt.float8e4)
        nc.sync.dma_start(wc_sb[:, :, :, :], wc_d[:, :])
        dma_group_inputs(1)
        fc1_sb = const.tile([128, CP, 2, MLP], dt.float8e4)
        nc.sync.dma_start(fc1_sb[:, :, :, :], fc1_d[:, :])
        fc2_sb = const.tile([128, MP, 2, C], dt.float8e4)
        nc.sync.dma_start(fc2_sb[:, :, :, :], fc2_d[:, :])

        ones16 = const.tile([128, 1], dt.float16)
        nc.vector.memset(ones16[:], 1.0)
        ones8 = const.tile([128, 2, 16], dt.float8e4)
        nc.vector.memset(ones8[:, :, :], 1.0)
        ones_row = const.tile([1, 128], dt.float16)
        nc.vector.memset(ones_row[:], 1.0)
        eps_ap = const.tile([1, 1], dt.float32)
        nc.vector.memset(eps_ap[:], float(EPS))

        def bcast_pack(pack_row, scr_idx, nslot, name):
            """Broadcast a packed [1, nslot*TG] fp16 SBUF row to a
            [128, nslot, TG] fp16 SBUF tile via a DRAM bounce: one DMA up,
            one DMA down whose DRAM-side AP has partition stride 0. Costs
            no PE/ACT/DVE/Pool time at all (pure DMA)."""
            bc = bcp.tile([128, nslot, TG], dt.float16, tag=f"bc{nslot}",
                          bufs=(3 if nslot == 2 else 2), name=f"bc_{name}")
            for k in range(nslot):
                bc_ps = psum.tile([128, TG], dt.float32, tag="st", bufs=3,
                                  name=f"bcps_{name}_{k}")
                nc.tensor.matmul(bc_ps[:], ones_row[0:1, :],
                                 pack_row[0:1, k * TG:(k + 1) * TG],
                                 start=True, stop=True)
                nc.vector.tensor_copy(bc[:, k, :], bc_ps[:])
            return bc

        def vec(idx, j):
            return vecs_sb[:, j * NV + idx: j * NV + idx + 1]

        x = [None] * NG            # (s, j) -> [128,TG] fp16 residual tiles
        st1 = [None] * NG          # S1: (bc [128,2,TG], {sfx: mrow})
        st2 = [None] * NG          # S2: bc [128,4,TG]
        stf = [None] * NG          # SF: bc [128,3,TG]
        h1g_all = [None] * NG      # norm1 fp8 hi/lo pairs
        apairs = [None] * NG       # (s, mp) -> [128,2,TG] fp8 gelu pairs
        f_uas = [None] * NG        # stage_F handoff: ua tiles

        def stage_L(g):
            """Blend the channel-major inputs into x0 (no transposes)."""
            xg = {}
            for j in range(CT):
                rj = in_tiles[g][0, j]
                dj = in_tiles[g][1, j]
                t1 = tmpp.tile([128, TG], dt.float16, tag="bl",
                               name=f"t1_{g}_{j}")
                nc.gpsimd.tensor_scalar(t1[:], dj[:], vec(V_A2, j),
                                        vec(V_A3, j), Alu.mult, Alu.add)
                x0r = xp.tile([128, TG], dt.float16, tag="res",
                              name=f"x0r_{g}_{j}")
                nc.vector.scalar_tensor_tensor(x0r[:], rj[:], vec(V_A1, j),
                                               t1[:], Alu.mult, Alu.add)
                t2 = tmpp.tile([128, TG], dt.float16, tag="bl",
                               name=f"t2_{g}_{j}")
                nc.gpsimd.tensor_scalar(t2[:], rj[:], vec(V_D2, j),
                                        vec(V_D3, j), Alu.mult, Alu.add)
                x0d = xp.tile([128, TG], dt.float16, tag="res",
                              name=f"x0d_{g}_{j}")
                nc.vector.scalar_tensor_tensor(x0d[:], dj[:], vec(V_D1, j),
                                               t2[:], Alu.mult, Alu.add)
                xg[0, j] = x0r
                xg[1, j] = x0d
            x[g] = xg

        def ln_stats(g, name, kind, scr_idx):
            """LN stats over channels for both streams of group g.

            kind: "n1" -> pack [rinv_r, rinv_d], also mrow bf16 per stream
                  "n2" -> pack [rinv_r, rinv_d, mrho_r, mrho_d]
                  "nf" -> pack [rinv_r, rinv_d, mrs]
            Returns (bc [128,nslot,TG] fp16 bcast tile, {sfx: mrow or None}).
            Sum streams in fp16 (6 mm); sumsq rides fp8 DoubleRow pairs
            (3 mm) written directly by the Pool square ops.
            """
            nslot = {"n1": 2, "n2": 4, "nf": 3}[kind]
            pack = rows.tile([1, 4 * TG], dt.float16, tag="pk", bufs=2,
                             name=f"pk_{name}")
            mrows = {}
            rinvs = {}
            stats = {}
            for s in (0, 1):
                sfx = "r" if s == 0 else "d"
                sq = []
                for j in range(CT):
                    sqt = sqp.tile([128, TG], dt.float16, tag="sq",
                                   name=f"sq_{name}_{s}_{j}")
                    nc.gpsimd.tensor_tensor(sqt[:], x[g][s, j][:],
                                            x[g][s, j][:], Alu.mult)
                    sq.append(sqt)
                stat = psum.tile([128, TG], dt.float32, tag="st", bufs=3,
                                 name=f"stat_{name}_{s}")
                # fp8 DoubleRow sq-sums cost ~0.3% rho error -> fails the
                # 2e-2 gate; keep both reductions in fp16.
                for j in range(CT):
                    nc.tensor.matmul(stat[32:33, :], ones16[:], x[g][s, j][:],
                                     tile_position=(0, 32),
                                     start=(j == 0), stop=(j == CT - 1))
                    nc.tensor.matmul(stat[0:1, :], ones16[:], sq[j][:],
                                     tile_position=(0, 0),
                                     start=(j == 0), stop=(j == CT - 1))
                stats[s] = stat
                sq1 = rows.tile([1, TG], dt.float32, tag="rowf",
                                name=f"sq1_{name}_{s}")
                nc.scalar.square(sq1[:], stat[32:33, :])
                u = rows.tile([1, TG], dt.float32, tag="rowf",
                              name=f"u_{name}_{s}")
                # u = C*var = sum(x^2) - (sum x)^2 / C
                nc.vector.scalar_tensor_tensor(u[:], sq1[:], -1.0 / C,
                                               stat[0:1, :], Alu.mult,
                                               Alu.add)
                # std16 = sqrt(u/C + eps) = sqrt(var + eps)   [1,TG] fp16
                std16 = rows.tile([1, TG], dt.float16, tag="rowh", bufs=6,
                                  name=f"std_{name}_{s}")
                nc.scalar.activation(std16[:], u[:], Act.Sqrt,
                                     bias=eps_ap[0:1, 0:1], scale=float(1.0 / C))
                rinv = pack[0:1, s * TG:(s + 1) * TG]
                with nc.allow_low_precision("fp16 LN reciprocal"):
                    nc.vector.reciprocal(rinv, std16[:])
                rinvs[s] = rinv
                mrows[sfx] = None
                if kind == "n1":
                    mrow = rows.tile([1, TG], dt.bfloat16, tag="rowh", bufs=6,
                                     name=f"mrow_{name}_{s}")
                    nc.vector.tensor_tensor(mrow[:], stat[32:33, :], rinv,
                                            Alu.mult)
                    mrows[sfx] = mrow
            if kind == "n2":
                for s in (0, 1):
                    # mrho = mean/sqrt(var+eps) = (sum x)/C * rinv
                    nc.vector.scalar_tensor_tensor(
                        pack[0:1, (2 + s) * TG:(3 + s) * TG],
                        stats[s][32:33, :], 1.0 / C, rinvs[s],
                        Alu.mult, Alu.mult)
            elif kind == "nf":
                # mrs = m_r*rho_r + m_d*rho_d
                t0 = rows.tile([1, TG], dt.float32, tag="rowf",
                               name=f"mr0_{name}")
                nc.vector.scalar_tensor_tensor(t0[:], stats[0][32:33, :],
                                               1.0 / C, rinvs[0],
                                               Alu.mult, Alu.mult)
                t1 = rows.tile([1, TG], dt.float32, tag="rowf",
                               name=f"mr1_{name}")
                nc.vector.scalar_tensor_tensor(t1[:], stats[1][32:33, :],
                                               1.0 / C, rinvs[1],
                                               Alu.mult, Alu.mult)
                nc.vector.tensor_tensor(pack[0:1, 2 * TG:3 * TG],
                                        t0[:], t1[:], Alu.add)
            bc = bcast_pack(pack, scr_idx, nslot, name)
            return bc, mrows

        def stage_W_apply(g):
            """norm1 apply: h1 = x0 * bcast(rinv), quantized fp8 hi/lo."""
            bc, _ = st1[g]
            hhig, hlog = {}, {}
            for s in (0, 1):
                ts_ = {}
                for kp in range(CP):
                    hhig[s, kp] = h1p.tile([128, 2, TG], dt.float8e4,
                                           tag="h1", name=f"h1hi_{g}_{s}_{kp}")
                    if not WC_SINGLE:
                        hlog[s, kp] = h1p.tile([128, 2, TG], dt.float8e4,
                                               tag="h1",
                                               name=f"h1lo_{g}_{s}_{kp}")
                for kp in range(CP):
                    for i in (0, 1):
                        j = 2 * kp + i
                        t_ = tmpp.tile([128, TG], dt.float16, tag="bl",
                                       name=f"h1t_{g}_{s}_{j}")
                        nc.gpsimd.tensor_tensor(t_[:], x[g][s, j][:],
                                                bc[:, s, :], Alu.mult)
                        if i == 0:
                            nc.scalar.copy(hhig[s, kp][:, i, :], t_[:])
                        else:
                            nc.vector.tensor_copy(hhig[s, kp][:, i, :], t_[:])
                        ts_[j] = t_
                if not WC_SINGLE:
                    for kp in range(CP):
                        for i in (0, 1):
                            nc.gpsimd.tensor_tensor(hlog[s, kp][:, i, :],
                                                    ts_[2 * kp + i][:],
                                                    hhig[s, kp][:, i, :],
                                                    Alu.subtract)
            h1g_all[g] = (hhig, hlog)

        def stage_W_mm(g, s):
            """Wc swap matmuls for source stream s -> writes x1 of the OTHER
            stream. Single-mo accumulation chains (2 psum slots max)."""
            hhig, hlog = h1g_all[g]
            _, mrows = st1[g]
            parts = (hhig,) if WC_SINGLE else (hhig, hlog)
            o = 1 - s
            mrow = mrows["r" if s == 0 else "d"]
            for mo in range(CT):
                acc = psum.tile([128, TG], dt.float32, tag="acc",
                                bufs=3, name=f"g_{g}_{s}_{mo}")
                first = True
                for part in parts:
                    for kp in range(CP):
                        nc.tensor.matmul(
                            acc[:],
                            wc_sb[:, kp, :, mo * 128:(mo + 1) * 128],
                            part[s, kp][:, :, :],
                            start=first and (kp == 0), stop=False,
                            perf_mode=PerfMode.DoubleRow)
                    first = False
                nc.tensor.matmul(
                    acc[:],
                    wcsum_sb[0:1, mo * 128:(mo + 1) * 128],
                    mrow[:], start=False, stop=True,
                    skip_group_check=True)
                # x1_o = acc/WSCALE + x0_o (pb == 0), o = other stream
                # (GPSIMD cannot access PSUM -> DVE)
                nc.vector.scalar_tensor_tensor(x[g][o, mo][:],
                                               acc[:],
                                               vec(V_SCLW, mo),
                                               x[g][o, mo][:],
                                               Alu.mult, Alu.add)

        def stage_Mloop(g):
            """norm2 apply + interleaved-stream fc1/GELU/fc2(co=0) loop."""
            bc4 = st2[g]
            h2g = {}
            for s in (0, 1):
                bc16 = bc4[:, s, :]
                bcm16 = bc4[:, 2 + s, :]
                for kp in range(CP):
                    pair = h2p.tile([128, 2, TG], dt.float8e4, tag="h2",
                                    name=f"h2_{g}_{s}_{kp}")
                    for i in (0, 1):
                        j = 2 * kp + i
                        t_ = tmpp.tile([128, TG], dt.float16, tag="bl",
                                       name=f"h2t_{g}_{s}_{j}")
                        nc.vector.tensor_tensor(t_[:], x[g][s, j][:], bc16,
                                                Alu.mult)
                        nc.gpsimd.tensor_tensor(pair[:, i, :], t_[:],
                                                bcm16, Alu.subtract)
                    h2g[s, kp] = pair
            # interleaved m-loop: ACT (gelu) and PE run concurrently; only
            # fc2 co=0 accumulates in-loop (psum pressure), rest in Mtail.
            acc0 = {}
            ap_g = {}
            apair_cur = {}
            for s in (0, 1):
                acc0[s] = psum.tile([128, TG], dt.float32, tag="acc", bufs=3,
                                    name=f"acc0_{g}_{s}")
            for m in range(MT):
                for s in (0, 1):
                    pf = psum.tile([128, TG], dt.float32, tag="ps", bufs=2,
                                   name=f"pf_{g}_{s}_{m}")
                    for kp in range(CP):
                        nc.tensor.matmul(
                            pf[:],
                            fc1_sb[:, kp, :, m * 128:(m + 1) * 128],
                            h2g[s, kp][:, :, :],
                            start=(kp == 0), stop=(kp == CP - 1),
                            perf_mode=PerfMode.DoubleRow)
                    if m % 2 == 0:
                        apair_cur[s] = a8p.tile([128, 2, TG], dt.float8e4,
                                                tag="a8",
                                                name=f"a_{g}_{s}_{m // 2}")
                        ap_g[s, m // 2] = apair_cur[s]
                    nc.scalar.activation(apair_cur[s][:, m % 2, :], pf[:],
                                         Act.Gelu, bias=fb1_sb[:, m:m + 1],
                                         scale=float(1.0 / WSCALE))
                    if m % 2 == 1:
                        mp = m // 2
                        nc.tensor.matmul(
                            acc0[s][:],
                            fc2_sb[:, mp, :, 0:128],
                            apair_cur[s][:, :, :],
                            start=(mp == 0), stop=(mp == MP - 1),
                            perf_mode=PerfMode.DoubleRow)
            apairs[g] = ap_g
            for s in (0, 1):
                nc.vector.scalar_tensor_tensor(x[g][s, 0][:], acc0[s][:],
                                               vec(V_SCL, 0), x[g][s, 0][:],
                                               Alu.mult, Alu.add)

        def stage_Mtail(g, cos):
            """fc2 output cols `cos` (subset of 1..5, both streams) swept
            densely from the persistent a8 pairs. Single-co accumulation
            chains keep at most 2 'acc' psum slots live."""
            ap_g = apairs[g]
            for s in (0, 1):
                for co in cos:
                    acc = psum.tile([128, TG], dt.float32, tag="acc",
                                    bufs=3, name=f"acc_{g}_{s}_{co}")
                    for mp in range(MP):
                        nc.tensor.matmul(
                            acc[:],
                            fc2_sb[:, mp, :, co * 128:(co + 1) * 128],
                            ap_g[s, mp][:, :, :],
                            start=(mp == 0), stop=(mp == MP - 1),
                            perf_mode=PerfMode.DoubleRow)
                    # (GPSIMD cannot access PSUM -> DVE)
                    nc.vector.scalar_tensor_tensor(x[g][s, co][:],
                                                   acc[:],
                                                   vec(V_SCL, co),
                                                   x[g][s, co][:],
                                                   Alu.mult, Alu.add)

        def stage_Fpre(g):
            """final-norm: ua = (x2r*rr + x2d*rd - mrs)*wfh + bf."""
            bc3 = stf[g]
            bc_rr = bc3[:, 0, :]
            bc_rd = bc3[:, 1, :]
            bc_mrs = bc3[:, 2, :]
            uas = []
            for j in range(CT):
                s1 = tmpp.tile([128, TG], dt.float16, tag="bl",
                               name=f"nf1_{g}_{j}")
                nc.vector.tensor_tensor(s1[:], x[g][0, j][:], bc_rr,
                                        Alu.mult)
                s2 = tmpp.tile([128, TG], dt.float16, tag="bl",
                               name=f"nf2_{g}_{j}")
                nc.gpsimd.tensor_tensor(s2[:], x[g][1, j][:], bc_rd,
                                        Alu.mult)
                nc.gpsimd.tensor_tensor(s1[:], s1[:], s2[:], Alu.add)
                nc.vector.tensor_tensor(s1[:], s1[:], bc_mrs,
                                        Alu.subtract)
                ua = uap.tile([128, TG], dt.float16, tag="uaff",
                              name=f"ua_{g}_{j}")
                nc.vector.tensor_scalar(ua[:], s1[:], vec(V_WFH, j),
                                        vec(V_BF, j), Alu.mult, Alu.add)
                uas.append(ua)
            f_uas[g] = uas

        def stage_Fout(g):
            """DMA the channel-major fp16 result tiles straight to HBM."""
            uas = f_uas[g]
            r0 = g * TG
            for j in range(CT):
                nc.sync.dma_start(
                    out_d[j * 128:(j + 1) * 128, r0:r0 + TG], uas[j][:])

        def S1(g):
            bc, mrows = ln_stats(g, f"n1_{g}", "n1", 3 * g + 0)
            st1[g] = (bc, mrows)

        def S2(g):
            bc, _ = ln_stats(g, f"n2_{g}", "n2", 3 * g + 1)
            st2[g] = bc

        def SF(g):
            bc, _ = ln_stats(g, f"nf_{g}", "nf", 3 * g + 2)
            stf[g] = bc

        # Deep software pipeline. PE executes (mostly) in emission order, so
        # every long non-PE chain (LN rows -> DMA broadcast -> norm apply)
        # is followed in the PE stream by independent matmul work: Mtail
        # chunks and other groups' stat reductions act as fillers. S1 runs
        # TWO groups ahead so W(g+1) finds its broadcast long since done.
        def emit(fn, *a):
            fn(*a)

        emit(stage_L, 0)
        emit(S1, 0)
        emit(stage_L, 1)
        emit(S1, 1)
        emit(stage_W_apply, 0)
        emit(stage_W_mm, 0, 0)
        emit(stage_W_mm, 0, 1)
        emit(S2, 0)
        for g in range(NG):
            if g + 2 < NG:
                dma_group_inputs(g + 2)
            emit(stage_Mloop, g)
            if g + 1 < NG:
                emit(stage_W_apply, g + 1)
            emit(stage_Mtail, g, (1, 2))
            if g + 1 < NG:
                emit(stage_W_mm, g + 1, 0)
            emit(stage_Mtail, g, (3, 4))
            if g + 1 < NG:
                emit(stage_W_mm, g + 1, 1)
            emit(stage_Mtail, g, (5,))
            emit(SF, g)
            if g + 1 < NG:
                emit(S2, g + 1)
            emit(stage_Fpre, g)
            emit(stage_Fout, g)
            # L/S1 for g+2 go AFTER Fpre(g) so only ~2 groups of residual
            # tiles are ever live; their stat matmuls still sit between
            # S2(g+1) and Mloop(g+1) in the PE stream (Fpre/Fout are PE-free)
            # and cover the S2 row-chain + broadcast latency.
            if g + 2 < NG:
                emit(stage_L, g + 2)
                emit(S1, g + 2)

    if legalize:
        _legalize_waits(nc)
    nc.finalize()
    return nc


def _legalize_waits(nc):
    """Walrus ISA structs have at most 1-2 sync-wait slots per instruction,
    but Tile's wait assignment can emit more. Move excess waits onto
    same-engine NoOps inserted immediately before the offending
    instruction."""
    import bass_rust
    nop_i = [0]
    for f in nc.m.functions:
        for b in f.blocks:
            insts = b.instructions
            out = []
            changed = False
            for ins in insts:
                si = getattr(ins, "sync_info", None)
                waits = list(si.on_wait) if (si and si.on_wait) else []
                if len(waits) > 1:
                    eng = ins.engine
                    for w in waits[:-1]:
                        n = bass_rust.InstNoOp(name=f"I-nopw-{nop_i[0]}")
                        nop_i[0] += 1
                        n.engine = eng
                        n.sync_info = bass_rust.SyncInfo(
                            on_wait=[w], on_update=[])
                        out.append(n)
                    ins.sync_info = bass_rust.SyncInfo(
                        on_wait=[waits[-1]], on_update=list(si.on_update or []))
                    changed = True
                out.append(ins)
            if changed:
                b.instructions = out


def _prepare(inputs):
    """Host-side folding: per-channel vectors + fused/packed weights."""
    f = lambda k: np.asarray(inputs[k], np.float64)
    alpha = f("alpha").reshape(C)

    s_r = f("bn_rgb_w") / np.sqrt(f("bn_rgb_var") + EPS)
    t_r = f("bn_rgb_b") - f("bn_rgb_mean") * s_r
    s_d = f("bn_depth_w") / np.sqrt(f("bn_depth_var") + EPS)
    t_d = f("bn_depth_b") - f("bn_depth_mean") * s_d

    w_r = np.asarray(inputs["bn_rgb_w"], np.float32)
    w_d = np.asarray(inputs["bn_depth_w"], np.float32)
    idx_r = np.argsort(np.abs(w_r), kind="stable")[:K_EX]
    idx_d = np.argsort(np.abs(w_d), kind="stable")[:K_EX]
    mask_r = np.zeros(C, bool)
    mask_r[idx_r] = True
    mask_d = np.zeros(C, bool)
    mask_d[idx_d] = True

    A1 = np.where(mask_r, alpha * s_r, s_r)
    A2 = np.where(mask_r, (1 - alpha) * s_d, 0.0)
    A3 = np.where(mask_r, alpha * t_r + (1 - alpha) * t_d, t_r)
    D1 = np.where(mask_d, alpha * s_d, s_d)
    D2 = np.where(mask_d, (1 - alpha) * s_r, 0.0)
    D3 = np.where(mask_d, alpha * t_d + (1 - alpha) * t_r, t_d)

    qkv_w = f("qkv_w")
    Wv = qkv_w[2 * C:, :]
    Wc = f("proj_w") @ Wv
    w1, b1 = f("norm1_w"), f("norm1_b")
    Wc_f = Wc * w1[None, :]
    pb = f("proj_b") + Wc @ b1
    wc_rowsum = Wc_f.sum(axis=1)

    w2, b2 = f("norm2_w"), f("norm2_b")
    fc1_f = f("fc1_w") * w2[None, :]
    fb1 = f("fc1_b") + f("fc1_w") @ b2
    fc2_w = f("fc2_w")
    fc2_b = f("fc2_b")
    assert np.allclose(fc2_b, 0.0), "kernel folds fc2_b==0 into V_SCL slot"
    wfh = 0.5 * f("normf_w")

    bf16 = ml_dtypes.bfloat16
    fp8 = ml_dtypes.float8_e4m3

    def pack_lhsT_pairs(wT, kp, m):
        # wT: [kp*256, m] -> [128, kp*2*m], [p, ((q*2+i)*m)+col] =
        #   wT[(2q+i)*128+p, col]   (DoubleRow k-pair layout)
        return np.ascontiguousarray(
            wT.reshape(kp, 2, 128, m).transpose(2, 0, 1, 3).reshape(
                128, kp * 2 * m))

    wc_pack = pack_lhsT_pairs(
        np.ascontiguousarray(Wc_f.T) * WSCALE, CP, C).astype(fp8)
    fc1_pack = pack_lhsT_pairs(
        np.ascontiguousarray(fc1_f.T) * WSCALE, CP, MLP).astype(fp8)
    fc2_pack = pack_lhsT_pairs(
        np.ascontiguousarray(fc2_w.T) * WSCALE, MP, C).astype(fp8)
    assert np.allclose(pb, 0.0), "kernel folds pb==0 into the Wc descale slot"

    scl = np.full(C, 1.0 / WSCALE)
    vv = [A1, A2, A3, D1, D2, D3, scl, scl, wfh, f("normf_b")]
    vecs = np.stack(vv, axis=-1).astype(np.float32)          # [C, NV]
    vecs = vecs.reshape(CT, 128, NV).transpose(1, 0, 2).reshape(128, CT * NV)
    vecs = np.ascontiguousarray(vecs)
    fb1_pack = np.ascontiguousarray(
        fb1.astype(np.float32).reshape(MT, 128).T)           # [128, MT]

    # mrow on device = C*mean/sqrt(var+eps); the rank-1 mean correction
    # accumulated into the Wc psum must equal -rowsum(Wc_f)*mean/sqrt(var+eps)
    # *WSCALE (descaled by V_SCLW later), so the wcsum lhsT carries /C.
    return {
        "wc": wc_pack,
        "fc1": fc1_pack,
        "fc2": fc2_pack,
        "vecs": vecs,
        "fb1": fb1_pack,
        "wcsum": (-wc_rowsum * WSCALE / C).astype(bf16).reshape(1, C),
    }


def _get_runner():
    """Build the Bass module once and cache a jitted shard_map executor."""
    if "runner" in _CACHE:
        return _CACHE["runner"]
    import jax
    from jax.sharding import Mesh, PartitionSpec
    from jax.experimental.shard_map import shard_map
    from concourse import bass2jax

    nc = _build_nc()
    bass2jax.install_neuronx_cc_hook()
    partition_name = (nc.partition_id_tensor.name
                      if nc.partition_id_tensor else None)
    in_names, out_names, out_avals = [], [], []
    for alloc in nc.m.functions[0].allocations:
        if not isinstance(alloc, mybir.MemoryLocationSet):
            continue
        name = alloc.memorylocations[0].name
        if alloc.kind == "ExternalInput":
            if name != partition_name:
                in_names.append(name)
        elif alloc.kind == "ExternalOutput":
            out_names.append(name)
            out_avals.append(jax.core.ShapedArray(
                tuple(alloc.tensor_shape), mybir.dt.np(alloc.dtype)))
    all_in_names = list(in_names) + list(out_names)
    if partition_name is not None:
        all_in_names.append(partition_name)

    def _body(*args):
        operands = list(args)
        if partition_name is not None:
            operands.append(bass2jax.partition_id_tensor())
        return tuple(bass2jax._bass_exec_p.bind(
            *operands, out_avals=tuple(out_avals),
            in_names=tuple(all_in_names), out_names=tuple(out_names),
            lowering_input_output_aliases=(),
            sim_require_finite=True, sim_require_nnan=True, nc=nc))

    devices = jax.devices()[:N_CORES]
    mesh = Mesh(np.asarray(devices), ("core",))
    sharded_args = {"rgb", "dep"}
    in_specs = tuple(
        PartitionSpec("core") if n in sharded_args else PartitionSpec()
        for n in in_names) + (PartitionSpec("core"),) * len(out_names)
    fn = jax.jit(
        shard_map(_body, mesh=mesh,
                  in_specs=in_specs,
                  out_specs=(PartitionSpec("core"),) * len(out_names),
                  check_rep=False),
        keep_unused=True)
    zeros = [jax.device_put(
        np.zeros((a.shape[0] * N_CORES,) + tuple(a.shape[1:]), a.dtype))
        for a in out_avals]
    _CACHE["runner"] = (fn, in_names, zeros, jax)
    return _CACHE["runner"]


def kernel(**inputs) -> np.ndarray:
    # channel-major per-core layout: [8*C, ROWS] fp16
    rgb = np.asarray(inputs["rgb"], np.float32).astype(np.float16)
    dep = np.asarray(inputs["depth"], np.float32).astype(np.float16)
    rgb_cm = np.ascontiguousarray(
        rgb.reshape(N_CORES, ROWS, C).transpose(0, 2, 1).reshape(
            N_CORES * C, ROWS))
    dep_cm = np.ascontiguousarray(
        dep.reshape(N_CORES, ROWS, C).transpose(0, 2, 1).reshape(
            N_CORES * C, ROWS))
    consts = _prepare(inputs)

    fn, in_names, zeros, jax = _get_runner()
    vals = {"rgb": rgb_cm, "dep": dep_cm}
    vals.update(consts)
    args = [vals[n] for n in in_names] + list(zeros)
    outs = fn(*args)
    out_cm = np.asarray(outs[0])                  # [8*C, ROWS] fp16
    out = out_cm.reshape(N_CORES, C, ROWS).transpose(0, 2, 1).reshape(B, T, C)
    return np.ascontiguousarray(out.astype(np.float32))


if __name__ == "__main__":
    print("built module ok" if _build_nc() else "")
